# revision 62
# baseline (speedup 1.0000x reference)
"""Trainium2 Bass kernel for blocked (compressed) multi-head attention.

Problem (hardcoded shapes):
    src [4, 4096, 1024] f32, H = 8 heads, dk = 128, local attention in
    blocks of 64 tokens, projections Wq/Wk/Wv/Wo [1024,1024] + biases.

Strategy:
    - 8-way data parallel over the 16384 tokens (2048 tokens/core; block and
      batch boundaries align, so cores are fully independent).
    - The q projection runs in fp8(e4m3) with DoubleRow perf mode (2x PE
      rate, contraction 256/matmul): host quantizes src*32 and Wq*256 to
      fp8 (exact power-of-two scales; descale folds into the exp scale).
      Quantization error flows through softmax only, measured 1.64e-2 total
      vs the 2e-2 gate. k/v/o stay bf16 (their errors hit the output
      directly; fp8 there measures 2.2e-2+).
    - The k bias is dropped entirely (softmax is invariant to the per-query
      constant q.bk adds to each score row); bv folds into bo on the host.
    - All inputs are host-prepacked so each DMA reads 4-8KiB contiguous
      per-partition runs. The input load is DMA-bandwidth-bound (~25us), so
      transfers are prioritized: the sync HWDGE queue carries (in FIFO
      order) fp8 src chunk 0, fp8 Wq, bq, bf16 src chunk 0, Wk, Wv; the
      gpsimd software DGE (own descriptor generator; the shared HWDGE
      generator costs ~0.63us per dma_start) carries ident/bo/Wo dep-gated
      on the Wk transfers, then the chunk-1 src prefetch. ~16 warmup
      matmuls on a zeroed tile cover the initial DMA wait and raise the PE
      p-state clock.
    - Per core, tokens are processed in chunks of 512, bf16 matmuls with
      fp32 PSUM accumulation: per 128-token block pair and head-group of 4:
      4 scores matmuls per group (both groups up front, launching both
      softmax chains early); Exp + softmax on the two 64-row halves write
      only the diagonal 64x64 quadrants of persistent probs tiles whose
      off-diagonal quadrants are zeroed once (cross-block probs exactly 0);
      probs transposed per head on the PE; PV with token-major v stationary
      gives attn^T (d-major); output projection out = attn^T.T @ Wo + bo.
    - PE bubble filling: the softmax chain latency (~1.5us per head-group)
      is covered by deferred work emitted between a group's scores and its
      transposes (dep-pinned so the scheduler can't hoist it): the output
      projection of pair gp-2, emitted in column halves; for the first two
      pairs of chunk 0 (nothing pending yet) the v projections of the last
      two pairs are deferred into those slots instead. The last pair's
      output DMA is split per column half on the sync queue to shorten the
      tail.
"""

import numpy as np
import ml_dtypes
from contextlib import ExitStack

import sys
import types

# Defensive: bass_utils imports antenv.axon_hooks when BASS_TRACE is set in
# the environment; provide a no-op hook module if the package is absent.
try:
    import antenv.axon_hooks  # noqa: F401
except ImportError:
    _anthooks = types.ModuleType("antenv.axon_hooks")
    _anthooks.get_axon_ntff_profile_hook = lambda: None
    _anthooks.set_axon_ntff_profile_hook = lambda h: None
    _antenv = sys.modules.setdefault("antenv", types.ModuleType("antenv"))
    _antenv.axon_hooks = _anthooks
    sys.modules.setdefault("antenv.axon_hooks", _anthooks)

import concourse.bass as bass
import concourse.tile as tile
from concourse import bacc, mybir
from concourse.bass_utils import run_bass_kernel_spmd

N_CORES = 8
B, S, D = 4, 4096, 1024
H, DK, BLOCK = 8, 128, 64
T_TOTAL = B * S
T_CORE = T_TOTAL // N_CORES   # 2048
NJ = D // 128                 # 8 column/row tiles of the weights
SCALE = 1.0 / float(np.sqrt(DK))
NWARM = 16
ODELAY = 2                    # out-projection emitted this many pairs late

BF16 = mybir.dt.bfloat16
F32 = mybir.dt.float32
FP8 = mybir.dt.float8e4
Exp = mybir.ActivationFunctionType.Exp
Copy = mybir.ActivationFunctionType.Copy
Mult = mybir.AluOpType.mult
DoubleRow = mybir.MatmulPerfMode.DoubleRow
# fp8 scales for the q projection (powers of two -> exact descale).
# q-path error through softmax stays ~1.6e-2 total (gate: 2e-2); see the
# host-side quantization in make_host_inputs.
SF_SRC = 32.0
SF_W = 256.0
SF_Q = SF_SRC * SF_W


def build_nc(t_core: int = T_CORE, chunk: int = 512) -> bacc.Bacc:
    assert t_core % chunk == 0 and chunk % 128 == 0
    nchunk = t_core // chunk
    pairs = chunk // 128            # 128-token block pairs per chunk
    nhalf = D // 512                # 512-wide output column groups
    npairs = t_core // 128

    nc = bacc.Bacc("TRN2", target_bir_lowering=False, debug=False,
                   num_devices=N_CORES)

    # All inputs are host-prepacked so every DMA reads 4-8KiB contiguous
    # per-partition runs (big descriptors -> full DMA bandwidth).
    srcc = nc.dram_tensor("srcc", [(t_core // chunk) * 128, NJ, chunk], BF16,
                          kind="ExternalInput").ap()
    srcq = nc.dram_tensor("srcq", [(t_core // chunk) * 128, NJ, chunk], FP8,
                          kind="ExternalInput").ap()
    wq8 = nc.dram_tensor("wq8", [4 * 128, 2, NJ, 128], FP8,
                         kind="ExternalInput").ap()
    wkg = nc.dram_tensor("wkg", [4 * 128, 2, NJ, 128], BF16,
                         kind="ExternalInput").ap()
    wvg = nc.dram_tensor("wvg", [2 * 128, 4, D], BF16,
                         kind="ExternalInput").ap()
    wog = nc.dram_tensor("wog", [2 * 128, 4, D], BF16,
                         kind="ExternalInput").ap()
    bqt = nc.dram_tensor("bqt", [128, NJ], F32, kind="ExternalInput").ap()
    bo = nc.dram_tensor("bo", [D], F32, kind="ExternalInput").ap()
    ident = nc.dram_tensor("ident", [128, 128], BF16, kind="ExternalInput").ap()
    out = nc.dram_tensor("out", [t_core, D], BF16, kind="ExternalOutput").ap()

    with tile.TileContext(nc) as tc, ExitStack() as ctx:
        const = ctx.enter_context(tc.tile_pool(name="const", bufs=1))
        srcp = ctx.enter_context(tc.tile_pool(name="srcp", bufs=3))
        qkp = ctx.enter_context(tc.tile_pool(name="qkp", bufs=2))
        vp = qkp
        attnp = srcp
        smp = ctx.enter_context(tc.tile_pool(name="smp", bufs=2))
        outp = ctx.enter_context(tc.tile_pool(name="outp", bufs=3))
        ps_proj = ctx.enter_context(tc.tile_pool(name="ps_proj", bufs=3, space="PSUM"))
        ps_sc = ctx.enter_context(tc.tile_pool(name="ps_sc", bufs=2, space="PSUM"))
        ps_ao = ps_sc
        ps_tr = ctx.enter_context(tc.tile_pool(name="ps_tr", bufs=1, space="PSUM"))

        # ---- PE warmup: matmuls on a zeroed tile fill the initial DMA wait
        # so the HAM clock-gate reaches 8/8 before real matmuls start.
        warm = const.tile([128, 512], BF16, tag="warm")
        nc.vector.memset(warm, 0.0)
        for _ in range(NWARM):
            wacc = ps_proj.tile([128, 512], F32, tag="acc")
            nc.tensor.matmul(wacc, warm[:, 0:128], warm, start=True, stop=True)

        # Persistent probs tiles: off-diagonal 64x64 quadrants stay zero for
        # the whole kernel; softmax writes only the diagonal quadrants.
        probs_t = []
        for b in range(2):
            pt = const.tile([128, H, 128], BF16, tag=f"probs{b}")
            nc.vector.memset(pt, 0.0)
            probs_t.append(pt)

        # ---- input DMAs. Two constraints drive the layout here:
        # (1) hardware-DGE descriptor generation serializes on ONE shared
        # HWDGE device (~0.63us per dma_start on sync/scalar), so second-
        # wave loads go out on the gpsimd software DGE (own generator);
        # (2) the DMA engines round-robin across all queued descriptors, so
        # the transfer waves are CHAINED with explicit dependency edges to
        # prioritize the critical path (src chunk 0 + Wq) over later-needed
        # tensors.
        def load_src_chunk(ci, eng=nc.scalar):
            t = srcp.tile([128, NJ, chunk], BF16, tag="s")
            d = eng.dma_start(out=t, in_=srcc[ci * 128:(ci + 1) * 128])
            return [t[:, i, :] for i in range(NJ)], d

        def load_srcq_chunk(ci, eng=nc.scalar):
            t = srcp.tile([128, NJ, chunk], FP8, tag="s8")
            d = eng.dma_start(out=t, in_=srcq[ci * 128:(ci + 1) * 128])
            return t, d

        def load_w_rows(name, src_r, eng, dmas):
            views = []
            for g in range(2):
                t = const.tile([128, 4, D], BF16, tag=f"{name}{g}")
                d = eng.dma_start(out=t,
                                  in_=src_r[g * 128:(g + 1) * 128])
                dmas.append(d)
                views.extend(t[:, i, :] for i in range(4))
            return views

        # waves 1+2 all on the sync HWDGE stream: per-queue FIFO order makes
        # issue order = transfer order, so priority needs no dep gates here:
        # fp8 src chunk 0, fp8 Wq, bq (first q-proj matmul), then bf16 src
        # chunk 0 and Wk (k-proj).
        s8_next, _ = load_srcq_chunk(0, nc.sync)
        wq_views, wq_dmas = [], []
        for g in range(4):
            t = const.tile([128, 2, NJ, 128], FP8, tag=f"wq{g}")
            wq_dmas.append(
                nc.sync.dma_start(out=t, in_=wq8[g * 128:(g + 1) * 128]))
            wq_views.append(t)
            if g == 0:
                bqt_sb = const.tile([128, NJ], F32, tag="bqt")
                nc.sync.dma_start(out=bqt_sb, in_=bqt)
        wq_sb = [t[:, j, :, :] for t in wq_views for j in range(2)]
        s_next, s0_dma = load_src_chunk(0, nc.sync)
        wk_views, wk_dmas = [], []
        for g in range(4):
            t = const.tile([128, 2, NJ, 128], BF16, tag=f"wk{g}")
            d = nc.sync.dma_start(out=t, in_=wkg[g * 128:(g + 1) * 128])
            wk_views.append(t)
            wk_dmas.append(d)
        wk_sb = [t[:, j, :, :] for t in wk_views for j in range(2)]
        # Wv continues the sync FIFO (needed at v-proj, right after Wk)
        wv_dmas = []
        wv_sb = load_w_rows("wv", wvg, nc.sync, wv_dmas)
        # wave 3 (gpsimd swDGE): ident, bo, Wo — needed from attention on
        wave3 = []
        ident_sb = const.tile([128, 128], BF16, tag="ident")
        nc.gpsimd.dma_start(out=ident_sb, in_=ident)
        bo_sb = const.tile([128, D], F32, tag="bo")
        bo_bc = bass.AP(tensor=bo.tensor, offset=bo.offset,
                        ap=[[0, 128], [1, D]])
        wave3.append(nc.gpsimd.dma_start(out=bo_sb, in_=bo_bc))
        wo_sb = load_w_rows("wo", wog, nc.gpsimd, wave3)
        wo_dma_last = wave3[-1]
        for d in wave3:
            bass._add_dep_helper(d.ins, wk_dmas[2].ins, sync=True,
                                 reason="dma wave 3 after wave 2")
        # wave 4: chunk-1 src prefetches on the gpsimd stream (no
        # head-of-line blocking risk there)
        s1_views, s1_dma = load_src_chunk(1, nc.gpsimd)
        bass._add_dep_helper(s1_dma.ins, wo_dma_last.ins, sync=True,
                             reason="src1 prefetch after wave 3")
        s81_t, s81_dma = load_srcq_chunk(1, nc.gpsimd)
        bass._add_dep_helper(s81_dma.ins, wo_dma_last.ins, sync=True,
                             reason="src1q prefetch after wave 3")

        # Deferred output projection: emitted ODELAY pairs late, one column
        # half after each head-group's attention, so the PE has matmul work
        # positioned exactly inside both softmax-latency bubbles of every
        # pair (including the last pair of each chunk).
        pending = {}

        def outproj_half(gp, n, split_dma=False):
            attn_t, c0, p, o_sb = pending[gp]
            acc = ps_proj.tile([128, 512], F32, tag="acc")
            first_mm = None
            for i in range(NJ):
                mm = nc.tensor.matmul(acc, attn_t[:, i, :],
                                      wo_sb[i][:, n * 512:(n + 1) * 512],
                                      start=(i == 0), stop=(i == NJ - 1))
                if first_mm is None:
                    first_mm = mm
            nc.vector.tensor_add(o_sb[:, n * 512:(n + 1) * 512], acc,
                                 bo_sb[:, n * 512:(n + 1) * 512])
            if split_dma:
                nc.sync.dma_start(
                    out=out[c0 + p * 128:c0 + (p + 1) * 128,
                            n * 512:(n + 1) * 512],
                    in_=o_sb[:, n * 512:(n + 1) * 512])
            return first_mm

        def finish_outproj(gp):
            attn_t, c0, p, o_sb = pending.pop(gp)
            eng = nc.sync if gp % 2 == 0 else nc.scalar
            eng.dma_start(
                out=out[c0 + p * 128:c0 + (p + 1) * 128, :], in_=o_sb)

        for ci in range(nchunk):
            c0 = ci * chunk
            s_sb = s_next
            s8_sb = s8_next
            # prefetch next chunk's src (chunk 1 was loaded in the preamble
            # wave chain)
            if ci == 0:
                s_next, s8_next = s1_views, s81_t
            elif ci + 1 < nchunk:
                s_next, _ = load_src_chunk(ci + 1, nc.sync)
                s8_next, _ = load_srcq_chunk(ci + 1, nc.sync)

            # ---- q projection (d-major) in fp8 DoubleRow: contraction 256
            # per matmul, 2x PE rate; PSUM carries scale SF_Q (descaled via
            # the exp activation scale). ----
            qt_sb, kt_sb = [], []
            for j in range(NJ):
                acc = ps_proj.tile([128, chunk], F32, tag="acc")
                for m in range(NJ // 2):
                    nc.tensor.matmul(acc, wq_sb[j][:, 2 * m:2 * m + 2, :],
                                     s8_sb[:, 2 * m:2 * m + 2, :],
                                     start=(m == 0), stop=(m == NJ // 2 - 1),
                                     perf_mode=DoubleRow)
                d = qkp.tile([128, chunk], BF16, tag=f"qt{j}")
                nc.vector.tensor_scalar_add(d, acc, bqt_sb[:, j:j + 1])
                qt_sb.append(d)
            # ---- k projection (bf16; the k bias is dropped entirely —
            # softmax is invariant to the per-query constant q.bk adds to
            # every score in a row — so k goes through a plain ACT copy).
            for j in range(NJ):
                acc = ps_proj.tile([128, chunk], F32, tag="acc")
                for i in range(NJ):
                    nc.tensor.matmul(acc, wk_sb[j][:, i, :], s_sb[i],
                                     start=(i == 0), stop=(i == NJ - 1))
                d = qkp.tile([128, chunk], BF16, tag=f"kt{j}")
                nc.scalar.activation(d, acc, Copy)
                kt_sb.append(d)

            # ---- v projection (token-major) ----
            v_sb = {}

            def vproj_half(t, n):
                if t not in v_sb:
                    vt = vp.tile([128, D], BF16, tag=f"v{t}")
                    v_sb[t] = vt
                vt = v_sb[t]
                acc = ps_proj.tile([128, 512], F32, tag="acc")
                first_mm = None
                for i in range(NJ):
                    mm = nc.tensor.matmul(acc,
                                          s_sb[i][:, t * 128:(t + 1) * 128],
                                          wv_sb[i][:, n * 512:(n + 1) * 512],
                                          start=(i == 0), stop=(i == NJ - 1))
                    if first_mm is None:
                        first_mm = mm
                nc.scalar.activation(vt[:, n * 512:(n + 1) * 512], acc, Copy)
                return first_mm

            # In chunk 0 the first ODELAY pairs have no deferred outproj to
            # fill their softmax bubbles, so the v projections of the LAST
            # two pairs are deferred into those bubbles instead; everything
            # else is projected upfront.
            upfront = pairs - ODELAY if ci == 0 else pairs
            for t in range(upfront):
                for n in range(nhalf):
                    vproj_half(t, n)

            # ---- per pair: attention for 2 head-groups ----
            for p in range(pairs):
                gp = ci * pairs + p
                pc = p * 128
                pt = probs_t[gp % 2]
                attn_t = attnp.tile([128, NJ, 128], BF16, tag=f"attn{gp % 2}")
                # Both head-groups' scores first, launching both softmax
                # chains on ACT/DVE as early as possible ...
                sc_last = []
                for hg in range(H // 4):
                    h0 = hg * 4
                    sc = ps_sc.tile([128, 512], F32, tag="sc")
                    for hh in range(4):
                        h = h0 + hh
                        hs = slice(hh * 128, (hh + 1) * 128)
                        mm = nc.tensor.matmul(sc[:, hs],
                                              qt_sb[h][:, pc:pc + 128],
                                              kt_sb[h][:, pc:pc + 128],
                                              start=(hh == 0), stop=(hh == 3))
                    sc_last.append(mm)
                    scv = sc.rearrange("p (a b) -> p a b", a=4)
                    # softmax on the two 64-row halves; only the diagonal
                    # 64x64 quadrant of each head's block is nonzero.
                    for lo in (0, 1):
                        rs = slice(lo * 64, lo * 64 + 64)
                        cs = slice(lo * 64, lo * 64 + 64)
                        exp_sb = smp.tile([64, 4, 64], F32, tag=f"exp{lo}")
                        nc.scalar.activation(exp_sb, scv[rs, :, cs],
                                             Exp, scale=SCALE / SF_Q)
                        den = smp.tile([64, 4], F32, tag=f"den{lo}")
                        nc.vector.reduce_sum(den, exp_sb,
                                             axis=mybir.AxisListType.X)
                        rcp = smp.tile([64, 4], F32, tag=f"rcp{lo}")
                        nc.vector.reciprocal(rcp, den)
                        nc.vector.tensor_tensor(
                            pt[rs, h0:h0 + 4, cs], exp_sb,
                            rcp.rearrange("p (a o) -> p a o", o=1)
                               .broadcast_to((64, 4, 64)),
                            op=Mult)
                # ... then per head-group: a deferred-outproj half as PE
                # filler inside the softmax-latency bubble (dep-pinned after
                # this pair's scores so the scheduler can't hoist it),
                # transposes, and the PV matmuls.
                for hg in range(H // 4):
                    h0 = hg * 4
                    if gp >= ODELAY:
                        fill = outproj_half(gp - ODELAY, hg)
                    else:
                        fill = vproj_half(pairs - ODELAY + gp, hg)
                    bass._add_dep_helper(fill.ins, sc_last[hg].ins,
                                         sync=True,
                                         reason="filler after scores")
                    trp = ps_tr.tile([128, 512], BF16, tag="trp")
                    for hh in range(4):
                        hs = slice(hh * 128, (hh + 1) * 128)
                        nc.tensor.transpose(trp[:, hs], pt[:, h0 + hh, :],
                                            ident_sb)
                    probsT = smp.tile([128, 512], BF16, tag="probsT")
                    nc.scalar.activation(probsT, trp, Copy)
                    ao = ps_ao.tile([128, 512], F32, tag="ao")
                    for hh in range(4):
                        h = h0 + hh
                        hs = slice(hh * 128, (hh + 1) * 128)
                        nc.tensor.matmul(ao[:, hs],
                                         v_sb[p][:, h * 128:(h + 1) * 128],
                                         probsT[:, hs], start=True, stop=True)
                    nc.scalar.activation(attn_t[:, h0:h0 + 4, :],
                                         ao.rearrange("p (a b) -> p a b", a=4),
                                         Copy)
                o_sb = outp.tile([128, D], BF16, tag="o")
                pending[gp] = (attn_t, c0, p, o_sb)
                if gp >= ODELAY:
                    finish_outproj(gp - ODELAY)

        for gp in range(npairs - ODELAY, npairs):
            for n in range(nhalf):
                outproj_half(gp, n, split_dma=True)
            pending.pop(gp)

    nc.compile()
    return nc


def make_host_inputs(src, Wq, bq, Wk, bk, Wv, bv, Wo, bo, t_core=T_CORE,
                     n_cores=N_CORES, chunk=512):
    """Prepare per-core input maps (host-side shard + transpose + bf16 cast).

    All tensors are packed so each DMA reads long contiguous per-partition
    runs (4-8KiB descriptors -> full DMA bandwidth):
      srcc [nchunk*128, NJ, chunk]  srcc[ci*128+p, j, t] = src_d-major[j*128+p,
                                    ci*chunk+t]
      wqg/wkg [4*128, 2, NJ, 128]   [...g*128+p, jj, i, c] = W[i*128+p,
                                    (2g+jj)*128+c]
      wvg/wog [2*128, 4, D]         [g*128+p, ii, n] = W[(4g+ii)*128+p, n]
    """
    bf = ml_dtypes.bfloat16
    f8 = ml_dtypes.float8_e4m3
    nchunk = t_core // chunk
    tokens = np.ascontiguousarray(np.asarray(src, dtype=np.float32)
                                  .reshape(-1, D))
    srct = np.ascontiguousarray(tokens.T).astype(bf)          # [D, T_total]
    srct8 = np.ascontiguousarray(
        (tokens.T * np.float32(SF_SRC))).astype(f8)           # fp8, x32

    def pack_qk(w, dt=bf, scale=1.0):
        # [D, D] -> [4*128, 2, NJ, 128] per the docstring above
        w4 = (np.asarray(w, np.float32) * np.float32(scale)
              ).reshape(NJ, 128, 4, 2, 128)
        # w4[i, p, g, jj, c] = W[i*128+p, ((2g+jj)*128)+c] ... need order
        # [g, p, jj, i, c]
        return np.ascontiguousarray(w4.transpose(2, 1, 3, 0, 4)
                                    .reshape(4 * 128, 2, NJ, 128)).astype(dt)

    def pack_rows(w):
        # [D, D] -> [2*128, 4, D]: [g*128+p, ii, n] = W[(4g+ii)*128+p, n]
        w4 = np.asarray(w, np.float32).reshape(2, 4, 128, D)
        return np.ascontiguousarray(w4.transpose(0, 2, 1, 3)
                                    .reshape(2 * 128, 4, D)).astype(bf)

    wq8 = pack_qk(Wq, dt=f8, scale=SF_W)
    wkg = pack_qk(Wk)
    wvg = pack_rows(Wv)
    wog = pack_rows(Wo)
    # q-proj PSUM carries scale SF_Q, so the q bias is pre-scaled to match
    bqt = np.ascontiguousarray(
        np.asarray(bq, np.float32).reshape(NJ, 128).T * np.float32(SF_Q))
    # bk is dropped: softmax is invariant to the constant q.bk adds along
    # each score row. probs rows sum to 1, so the v-bias folds into the
    # output bias:  out = (attn0 + bv)@Wo + bo = attn0@Wo + (bv@Wo + bo)
    bof = (np.asarray(bo, np.float64)
           + np.asarray(bv, np.float64) @ np.asarray(Wo, np.float64)
           ).astype(np.float32)
    ident = np.eye(128, dtype=np.float32).astype(bf)
    in_maps = []
    for c in range(n_cores):
        def chunked(st):
            s4 = np.asarray(st[:, c * t_core:(c + 1) * t_core]
                            ).reshape(NJ, 128, nchunk, chunk)
            return np.ascontiguousarray(s4.transpose(2, 1, 0, 3)
                                        .reshape(nchunk * 128, NJ, chunk))
        in_maps.append({
            "srcc": chunked(srct), "srcq": chunked(srct8),
            "wq8": wq8, "wkg": wkg, "wvg": wvg, "wog": wog,
            "bqt": bqt, "bo": bof,
            "ident": ident,
        })
    return in_maps


_NC_CACHE = {}


def _get_nc():
    if "nc" not in _NC_CACHE:
        _NC_CACHE["nc"] = build_nc()
    return _NC_CACHE["nc"]


def run_on_hw(in_maps, **kwargs):
    nc = _get_nc()
    return run_bass_kernel_spmd(nc, in_maps, core_ids=list(range(N_CORES)),
                                **kwargs)


def kernel(src, Wq, bq, Wk, bk, Wv, bv, Wo, bo):
    in_maps = make_host_inputs(src, Wq, bq, Wk, bk, Wv, bv, Wo, bo)
    res = run_on_hw(in_maps)
    out = np.concatenate([res.results[c]["out"] for c in range(N_CORES)],
                         axis=0)
    return out.reshape(B, S, D).astype(np.float32)



# revision 63
# speedup vs baseline: 1.1961x; 1.1961x over previous
"""Trainium2 Bass kernel for blocked (compressed) multi-head attention.

Problem (hardcoded shapes):
    src [4, 4096, 1024] f32, H = 8 heads, dk = 128, local attention in
    blocks of 64 tokens, projections Wq/Wk/Wv/Wo [1024,1024] + biases.

Strategy:
    - 8-way data parallel over the 16384 tokens (2048 tokens/core; block and
      batch boundaries align, so cores are fully independent).
    - The q projection runs in fp8(e4m3) with DoubleRow perf mode (2x PE
      rate, contraction 256/matmul): host quantizes src*32 and Wq*256 to
      fp8 (exact power-of-two scales; descale folds into the exp scale).
      Quantization error flows through softmax only, measured 1.64e-2 total
      vs the 2e-2 gate. k/v/o stay bf16 (their errors hit the output
      directly; fp8 there measures 2.2e-2+).
    - The k bias is dropped entirely (softmax is invariant to the per-query
      constant q.bk adds to each score row); bv folds into bo on the host.
    - All inputs are host-prepacked so each DMA reads 4-8KiB contiguous
      per-partition runs. The input load is DMA-bandwidth-bound (~25us), so
      transfers are prioritized: the sync HWDGE queue carries (in FIFO
      order) fp8 src chunk 0, fp8 Wq, bq, bf16 src chunk 0, Wk, Wv; the
      gpsimd software DGE (own descriptor generator; the shared HWDGE
      generator costs ~0.63us per dma_start) carries ident/bo/Wo dep-gated
      on the Wk transfers, then the chunk-1 src prefetch. ~16 warmup
      matmuls on a zeroed tile cover the initial DMA wait and raise the PE
      p-state clock.
    - Per core, tokens are processed in chunks of 512, bf16 matmuls with
      fp32 PSUM accumulation: per 128-token block pair and head-group of 4:
      4 scores matmuls per group (both groups up front, launching both
      softmax chains early); Exp + softmax on the two 64-row halves write
      only the diagonal 64x64 quadrants of persistent probs tiles whose
      off-diagonal quadrants are zeroed once (cross-block probs exactly 0);
      probs transposed per head on the PE; PV with token-major v stationary
      gives attn^T (d-major); output projection out = attn^T.T @ Wo + bo.
    - PE bubble filling: the softmax chain latency (~1.5us per head-group)
      is covered by deferred work emitted between a group's scores and its
      transposes (dep-pinned so the scheduler can't hoist it): the output
      projection of pair gp-2, emitted in column halves; for the first two
      pairs of chunk 0 (nothing pending yet) the v projections of the last
      two pairs are deferred into those slots instead. The last pair's
      output DMA is split per column half on the sync queue to shorten the
      tail.
"""

import numpy as np
import ml_dtypes
from contextlib import ExitStack

import sys
import types

# Defensive: bass_utils imports antenv.axon_hooks when BASS_TRACE is set in
# the environment; provide a no-op hook module if the package is absent.
try:
    import antenv.axon_hooks  # noqa: F401
except ImportError:
    _anthooks = types.ModuleType("antenv.axon_hooks")
    _anthooks.get_axon_ntff_profile_hook = lambda: None
    _anthooks.set_axon_ntff_profile_hook = lambda h: None
    _antenv = sys.modules.setdefault("antenv", types.ModuleType("antenv"))
    _antenv.axon_hooks = _anthooks
    sys.modules.setdefault("antenv.axon_hooks", _anthooks)

import concourse.bass as bass
import concourse.tile as tile
from concourse import bacc, mybir
from concourse.bass_utils import run_bass_kernel_spmd

N_CORES = 8
B, S, D = 4, 4096, 1024
H, DK, BLOCK = 8, 128, 64
T_TOTAL = B * S
T_CORE = T_TOTAL // N_CORES   # 2048
NJ = D // 128                 # 8 column/row tiles of the weights
SCALE = 1.0 / float(np.sqrt(DK))
NWARM = 16
ODELAY = 2                    # out-projection emitted this many pairs late

BF16 = mybir.dt.bfloat16
F32 = mybir.dt.float32
FP8 = mybir.dt.float8e4
Exp = mybir.ActivationFunctionType.Exp
Copy = mybir.ActivationFunctionType.Copy
Mult = mybir.AluOpType.mult
DoubleRow = mybir.MatmulPerfMode.DoubleRow
# fp8 scales for the q projection (powers of two -> exact descale).
# q-path error through softmax stays ~1.6e-2 total (gate: 2e-2); see the
# host-side quantization in make_host_inputs.
SF_SRC = 32.0
SF_W = 256.0
SF_Q = SF_SRC * SF_W


def build_nc(t_core: int = T_CORE, chunk: int = 512) -> bacc.Bacc:
    assert t_core % chunk == 0 and chunk % 128 == 0
    nchunk = t_core // chunk
    pairs = chunk // 128            # 128-token block pairs per chunk
    nhalf = D // 512                # 512-wide output column groups
    npairs = t_core // 128

    nc = bacc.Bacc("TRN2", target_bir_lowering=False, debug=False,
                   num_devices=N_CORES)

    # All inputs are host-prepacked so every DMA reads 4-8KiB contiguous
    # per-partition runs (big descriptors -> full DMA bandwidth).
    srcc = nc.dram_tensor("srcc", [(t_core // chunk) * 128, NJ, chunk], BF16,
                          kind="ExternalInput").ap()
    srcq = nc.dram_tensor("srcq", [(t_core // chunk) * 128, NJ, chunk], FP8,
                          kind="ExternalInput").ap()
    wq8 = nc.dram_tensor("wq8", [4 * 128, 2, NJ, 128], FP8,
                         kind="ExternalInput").ap()
    wkg = nc.dram_tensor("wkg", [4 * 128, 2, NJ, 128], BF16,
                         kind="ExternalInput").ap()
    wvg = nc.dram_tensor("wvg", [2 * 128, 4, D], BF16,
                         kind="ExternalInput").ap()
    wog = nc.dram_tensor("wog", [2 * 128, 4, D], BF16,
                         kind="ExternalInput").ap()
    bqt = nc.dram_tensor("bqt", [128, NJ], F32, kind="ExternalInput").ap()
    bo = nc.dram_tensor("bo", [D], F32, kind="ExternalInput").ap()
    ident = nc.dram_tensor("ident", [128, 128], BF16, kind="ExternalInput").ap()
    out = nc.dram_tensor("out", [t_core, D], BF16, kind="ExternalOutput").ap()

    with tile.TileContext(nc) as tc, ExitStack() as ctx:
        const = ctx.enter_context(tc.tile_pool(name="const", bufs=1))
        srcp = ctx.enter_context(tc.tile_pool(name="srcp", bufs=3))
        qkp = ctx.enter_context(tc.tile_pool(name="qkp", bufs=2))
        vp = qkp
        attnp = srcp
        smp = ctx.enter_context(tc.tile_pool(name="smp", bufs=4))
        outp = smp
        ps_proj = ctx.enter_context(tc.tile_pool(name="ps_proj", bufs=3, space="PSUM"))
        ps_sc = ctx.enter_context(tc.tile_pool(name="ps_sc", bufs=2, space="PSUM"))
        ps_ao = ps_sc
        ps_tr = ctx.enter_context(tc.tile_pool(name="ps_tr", bufs=1, space="PSUM"))

        # ---- PE warmup: matmuls on a zeroed tile fill the initial DMA wait
        # so the HAM clock-gate reaches 8/8 before real matmuls start.
        warm = const.tile([128, 512], BF16, tag="warm")
        nc.vector.memset(warm, 0.0)
        for _ in range(NWARM):
            wacc = ps_proj.tile([128, 512], F32, tag="acc")
            nc.tensor.matmul(wacc, warm[:, 0:128], warm, start=True, stop=True)

        # Persistent probs tiles: off-diagonal 64x64 quadrants stay zero for
        # the whole kernel; softmax writes only the diagonal quadrants.
        probs_t = []
        for b in range(2):
            pt = const.tile([128, H, 128], BF16, tag=f"probs{b}")
            nc.vector.memset(pt, 0.0)
            probs_t.append(pt)

        # ---- input DMAs. Two constraints drive the layout here:
        # (1) hardware-DGE descriptor generation serializes on ONE shared
        # HWDGE device (~0.63us per dma_start on sync/scalar), so second-
        # wave loads go out on the gpsimd software DGE (own generator);
        # (2) the DMA engines round-robin across all queued descriptors, so
        # the transfer waves are CHAINED with explicit dependency edges to
        # prioritize the critical path (src chunk 0 + Wq) over later-needed
        # tensors.
        def load_src_chunk(ci, eng=nc.scalar):
            t = srcp.tile([128, NJ, chunk], BF16, tag="s")
            d = eng.dma_start(out=t, in_=srcc[ci * 128:(ci + 1) * 128])
            return [t[:, i, :] for i in range(NJ)], d

        def load_srcq_chunk(ci, eng=nc.scalar):
            t = srcp.tile([128, NJ, chunk], FP8, tag="s8")
            d = eng.dma_start(out=t, in_=srcq[ci * 128:(ci + 1) * 128])
            return t, d

        def load_w_rows(name, src_r, eng, dmas):
            views = []
            for g in range(2):
                t = const.tile([128, 4, D], BF16, tag=f"{name}{g}")
                d = eng.dma_start(out=t,
                                  in_=src_r[g * 128:(g + 1) * 128])
                dmas.append(d)
                views.extend(t[:, i, :] for i in range(4))
            return views

        # waves 1+2 all on the sync HWDGE stream: per-queue FIFO order makes
        # issue order = transfer order, so priority needs no dep gates here:
        # fp8 src chunk 0, fp8 Wq, bq (first q-proj matmul), then bf16 src
        # chunk 0 and Wk (k-proj).
        s8_next, _ = load_srcq_chunk(0, nc.sync)
        wq_views, wq_dmas = [], []
        for g in range(4):
            t = const.tile([128, 2, NJ, 128], FP8, tag=f"wq{g}")
            wq_dmas.append(
                nc.sync.dma_start(out=t, in_=wq8[g * 128:(g + 1) * 128]))
            wq_views.append(t)
            if g == 0:
                bqt_sb = const.tile([128, NJ], F32, tag="bqt")
                nc.sync.dma_start(out=bqt_sb, in_=bqt)
        wq_sb = [t[:, j, :, :] for t in wq_views for j in range(2)]
        s_next, s0_dma = load_src_chunk(0, nc.sync)
        wk_views, wk_dmas = [], []
        for g in range(4):
            t = const.tile([128, 2, NJ, 128], BF16, tag=f"wk{g}")
            d = nc.sync.dma_start(out=t, in_=wkg[g * 128:(g + 1) * 128])
            wk_views.append(t)
            wk_dmas.append(d)
        wk_sb = [t[:, j, :, :] for t in wk_views for j in range(2)]
        # Wv continues the sync FIFO (needed at v-proj, right after Wk)
        wv_dmas = []
        wv_sb = load_w_rows("wv", wvg, nc.sync, wv_dmas)
        # wave 3 (gpsimd swDGE): ident, bo, Wo — needed from attention on
        wave3 = []
        ident_sb = const.tile([128, 128], BF16, tag="ident")
        nc.gpsimd.dma_start(out=ident_sb, in_=ident)
        bo_sb = const.tile([128, D], F32, tag="bo")
        bo_bc = bass.AP(tensor=bo.tensor, offset=bo.offset,
                        ap=[[0, 128], [1, D]])
        wave3.append(nc.gpsimd.dma_start(out=bo_sb, in_=bo_bc))
        wo_sb = load_w_rows("wo", wog, nc.gpsimd, wave3)
        wo_dma_last = wave3[-1]
        for d in wave3:
            bass._add_dep_helper(d.ins, wk_dmas[2].ins, sync=True,
                                 reason="dma wave 3 after wave 2")
        # wave 4: chunk-1 src prefetches on the gpsimd stream (no
        # head-of-line blocking risk there)
        s1_views, s1_dma = load_src_chunk(1, nc.gpsimd)
        bass._add_dep_helper(s1_dma.ins, wo_dma_last.ins, sync=True,
                             reason="src1 prefetch after wave 3")
        s81_t, s81_dma = load_srcq_chunk(1, nc.gpsimd)
        bass._add_dep_helper(s81_dma.ins, wo_dma_last.ins, sync=True,
                             reason="src1q prefetch after wave 3")

        # Deferred output projection: emitted ODELAY pairs late, one column
        # half after each head-group's attention, so the PE has matmul work
        # positioned exactly inside both softmax-latency bubbles of every
        # pair (including the last pair of each chunk).
        pending = {}

        def outproj_half(gp, n, split_dma=False):
            attn_t, c0, p, o_sb = pending[gp]
            acc = ps_proj.tile([128, 512], F32, tag="acc")
            first_mm = None
            for i in range(NJ):
                mm = nc.tensor.matmul(acc, attn_t[:, i, :],
                                      wo_sb[i][:, n * 512:(n + 1) * 512],
                                      start=(i == 0), stop=(i == NJ - 1))
                if first_mm is None:
                    first_mm = mm
            nc.vector.tensor_add(o_sb[:, n * 512:(n + 1) * 512], acc,
                                 bo_sb[:, n * 512:(n + 1) * 512])
            if split_dma:
                nc.sync.dma_start(
                    out=out[c0 + p * 128:c0 + (p + 1) * 128,
                            n * 512:(n + 1) * 512],
                    in_=o_sb[:, n * 512:(n + 1) * 512])
            return first_mm

        def finish_outproj(gp):
            attn_t, c0, p, o_sb = pending.pop(gp)
            eng = nc.sync if gp % 2 == 0 else nc.scalar
            eng.dma_start(
                out=out[c0 + p * 128:c0 + (p + 1) * 128, :], in_=o_sb)

        for ci in range(nchunk):
            c0 = ci * chunk
            s_sb = s_next
            s8_sb = s8_next
            # prefetch next chunk's src (chunk 1 was loaded in the preamble
            # wave chain)
            if ci == 0:
                s_next, s8_next = s1_views, s81_t
            elif ci + 1 < nchunk:
                s_next, _ = load_src_chunk(ci + 1, nc.sync)
                s8_next, _ = load_srcq_chunk(ci + 1, nc.sync)

            # ---- q projection (d-major) in fp8 DoubleRow: contraction 256
            # per matmul, 2x PE rate; PSUM carries scale SF_Q (descaled via
            # the exp activation scale). ----
            qt_sb, kt_sb = [], []
            for j in range(NJ):
                acc = ps_proj.tile([128, chunk], F32, tag="acc")
                for m in range(NJ // 2):
                    nc.tensor.matmul(acc, wq_sb[j][:, 2 * m:2 * m + 2, :],
                                     s8_sb[:, 2 * m:2 * m + 2, :],
                                     start=(m == 0), stop=(m == NJ // 2 - 1),
                                     perf_mode=DoubleRow)
                d = qkp.tile([128, chunk], BF16, tag=f"qt{j}")
                nc.vector.tensor_scalar_add(d, acc, bqt_sb[:, j:j + 1])
                qt_sb.append(d)
            # ---- k projection (bf16; the k bias is dropped entirely —
            # softmax is invariant to the per-query constant q.bk adds to
            # every score in a row — so k goes through a plain ACT copy).
            for j in range(NJ):
                acc = ps_proj.tile([128, chunk], F32, tag="acc")
                for i in range(NJ):
                    nc.tensor.matmul(acc, wk_sb[j][:, i, :], s_sb[i],
                                     start=(i == 0), stop=(i == NJ - 1))
                d = qkp.tile([128, chunk], BF16, tag=f"kt{j}")
                nc.scalar.activation(d, acc, Copy)
                kt_sb.append(d)

            # ---- v projection (token-major) ----
            v_sb = {}

            def vproj_half(t, n):
                if t not in v_sb:
                    vt = vp.tile([128, D], BF16, tag=f"v{t}")
                    v_sb[t] = vt
                vt = v_sb[t]
                acc = ps_proj.tile([128, 512], F32, tag="acc")
                first_mm = None
                for i in range(NJ):
                    mm = nc.tensor.matmul(acc,
                                          s_sb[i][:, t * 128:(t + 1) * 128],
                                          wv_sb[i][:, n * 512:(n + 1) * 512],
                                          start=(i == 0), stop=(i == NJ - 1))
                    if first_mm is None:
                        first_mm = mm
                nc.scalar.activation(vt[:, n * 512:(n + 1) * 512], acc, Copy)
                return first_mm

            # In chunk 0 the first ODELAY pairs have no deferred outproj to
            # fill their softmax bubbles, so the v projections of the LAST
            # two pairs are deferred into those bubbles instead; everything
            # else is projected upfront.
            upfront = pairs - ODELAY if ci == 0 else pairs
            for t in range(upfront):
                for n in range(nhalf):
                    vproj_half(t, n)

            # ---- per pair: attention for 2 head-groups ----
            for p in range(pairs):
                gp = ci * pairs + p
                pc = p * 128
                pt = probs_t[gp % 2]
                attn_t = attnp.tile([128, NJ, 128], BF16, tag=f"attn{gp % 2}")
                # Both head-groups' scores first, launching both softmax
                # chains on ACT/DVE as early as possible ...
                sc_last = []
                for hg in range(H // 4):
                    h0 = hg * 4
                    sc = ps_sc.tile([128, 512], F32, tag="sc")
                    for hh in range(4):
                        h = h0 + hh
                        hs = slice(hh * 128, (hh + 1) * 128)
                        mm = nc.tensor.matmul(sc[:, hs],
                                              qt_sb[h][:, pc:pc + 128],
                                              kt_sb[h][:, pc:pc + 128],
                                              start=(hh == 0), stop=(hh == 3))
                    sc_last.append(mm)
                    scv = sc.rearrange("p (a b) -> p a b", a=4)
                    # softmax on the two 64-row halves; only the diagonal
                    # 64x64 quadrant of each head's block is nonzero.
                    for lo in (0, 1):
                        rs = slice(lo * 64, lo * 64 + 64)
                        cs = slice(lo * 64, lo * 64 + 64)
                        exp_sb = smp.tile([64, 4, 64], F32, tag=f"exp{lo}")
                        nc.scalar.activation(exp_sb, scv[rs, :, cs],
                                             Exp, scale=SCALE / SF_Q)
                        den = smp.tile([64, 4], F32, tag=f"den{lo}")
                        nc.vector.reduce_sum(den, exp_sb,
                                             axis=mybir.AxisListType.X)
                        rcp = smp.tile([64, 4], F32, tag=f"rcp{lo}")
                        nc.vector.reciprocal(rcp, den)
                        nc.vector.tensor_tensor(
                            pt[rs, h0:h0 + 4, cs], exp_sb,
                            rcp.rearrange("p (a o) -> p a o", o=1)
                               .broadcast_to((64, 4, 64)),
                            op=Mult)
                # ... then per head-group: a deferred-outproj half as PE
                # filler inside the softmax-latency bubble (dep-pinned after
                # this pair's scores so the scheduler can't hoist it),
                # transposes, and the PV matmuls.
                for hg in range(H // 4):
                    h0 = hg * 4
                    if gp >= ODELAY:
                        fill = outproj_half(gp - ODELAY, hg)
                    else:
                        fill = vproj_half(pairs - ODELAY + gp, hg)
                    bass._add_dep_helper(fill.ins, sc_last[hg].ins,
                                         sync=True,
                                         reason="filler after scores")
                    trp = ps_tr.tile([128, 512], BF16, tag="trp")
                    for hh in range(4):
                        hs = slice(hh * 128, (hh + 1) * 128)
                        nc.tensor.transpose(trp[:, hs], pt[:, h0 + hh, :],
                                            ident_sb)
                    probsT = smp.tile([128, 512], BF16, tag="probsT")
                    nc.scalar.activation(probsT, trp, Copy)
                    ao = ps_ao.tile([128, 512], F32, tag="ao")
                    for hh in range(4):
                        h = h0 + hh
                        hs = slice(hh * 128, (hh + 1) * 128)
                        nc.tensor.matmul(ao[:, hs],
                                         v_sb[p][:, h * 128:(h + 1) * 128],
                                         probsT[:, hs], start=True, stop=True)
                    nc.scalar.activation(attn_t[:, h0:h0 + 4, :],
                                         ao.rearrange("p (a b) -> p a b", a=4),
                                         Copy)
                o_sb = outp.tile([128, D], BF16, tag="o")
                pending[gp] = (attn_t, c0, p, o_sb)
                if gp >= ODELAY:
                    finish_outproj(gp - ODELAY)

        for gp in range(npairs - ODELAY, npairs):
            for n in range(nhalf):
                outproj_half(gp, n, split_dma=True)
            pending.pop(gp)

    nc.compile()
    return nc


def make_host_inputs(src, Wq, bq, Wk, bk, Wv, bv, Wo, bo, t_core=T_CORE,
                     n_cores=N_CORES, chunk=512):
    """Prepare per-core input maps (host-side shard + transpose + bf16 cast).

    All tensors are packed so each DMA reads long contiguous per-partition
    runs (4-8KiB descriptors -> full DMA bandwidth):
      srcc [nchunk*128, NJ, chunk]  srcc[ci*128+p, j, t] = src_d-major[j*128+p,
                                    ci*chunk+t]
      wqg/wkg [4*128, 2, NJ, 128]   [...g*128+p, jj, i, c] = W[i*128+p,
                                    (2g+jj)*128+c]
      wvg/wog [2*128, 4, D]         [g*128+p, ii, n] = W[(4g+ii)*128+p, n]
    """
    bf = ml_dtypes.bfloat16
    f8 = ml_dtypes.float8_e4m3
    nchunk = t_core // chunk
    tokens = np.ascontiguousarray(np.asarray(src, dtype=np.float32)
                                  .reshape(-1, D))
    srct = np.ascontiguousarray(tokens.T).astype(bf)          # [D, T_total]
    srct8 = np.ascontiguousarray(
        (tokens.T * np.float32(SF_SRC))).astype(f8)           # fp8, x32

    def pack_qk(w, dt=bf, scale=1.0):
        # [D, D] -> [4*128, 2, NJ, 128] per the docstring above
        w4 = (np.asarray(w, np.float32) * np.float32(scale)
              ).reshape(NJ, 128, 4, 2, 128)
        # w4[i, p, g, jj, c] = W[i*128+p, ((2g+jj)*128)+c] ... need order
        # [g, p, jj, i, c]
        return np.ascontiguousarray(w4.transpose(2, 1, 3, 0, 4)
                                    .reshape(4 * 128, 2, NJ, 128)).astype(dt)

    def pack_rows(w):
        # [D, D] -> [2*128, 4, D]: [g*128+p, ii, n] = W[(4g+ii)*128+p, n]
        w4 = np.asarray(w, np.float32).reshape(2, 4, 128, D)
        return np.ascontiguousarray(w4.transpose(0, 2, 1, 3)
                                    .reshape(2 * 128, 4, D)).astype(bf)

    wq8 = pack_qk(Wq, dt=f8, scale=SF_W)
    wkg = pack_qk(Wk)
    wvg = pack_rows(Wv)
    wog = pack_rows(Wo)
    # q-proj PSUM carries scale SF_Q, so the q bias is pre-scaled to match
    bqt = np.ascontiguousarray(
        np.asarray(bq, np.float32).reshape(NJ, 128).T * np.float32(SF_Q))
    # bk is dropped: softmax is invariant to the constant q.bk adds along
    # each score row. probs rows sum to 1, so the v-bias folds into the
    # output bias:  out = (attn0 + bv)@Wo + bo = attn0@Wo + (bv@Wo + bo)
    bof = (np.asarray(bo, np.float64)
           + np.asarray(bv, np.float64) @ np.asarray(Wo, np.float64)
           ).astype(np.float32)
    ident = np.eye(128, dtype=np.float32).astype(bf)
    in_maps = []
    for c in range(n_cores):
        def chunked(st):
            s4 = np.asarray(st[:, c * t_core:(c + 1) * t_core]
                            ).reshape(NJ, 128, nchunk, chunk)
            return np.ascontiguousarray(s4.transpose(2, 1, 0, 3)
                                        .reshape(nchunk * 128, NJ, chunk))
        in_maps.append({
            "srcc": chunked(srct), "srcq": chunked(srct8),
            "wq8": wq8, "wkg": wkg, "wvg": wvg, "wog": wog,
            "bqt": bqt, "bo": bof,
            "ident": ident,
        })
    return in_maps


_NC_CACHE = {}


def _get_nc():
    if "nc" not in _NC_CACHE:
        _NC_CACHE["nc"] = build_nc()
    return _NC_CACHE["nc"]


def run_on_hw(in_maps, **kwargs):
    nc = _get_nc()
    return run_bass_kernel_spmd(nc, in_maps, core_ids=list(range(N_CORES)),
                                **kwargs)


def kernel(src, Wq, bq, Wk, bk, Wv, bv, Wo, bo):
    in_maps = make_host_inputs(src, Wq, bq, Wk, bk, Wv, bv, Wo, bo)
    res = run_on_hw(in_maps)
    out = np.concatenate([res.results[c]["out"] for c in range(N_CORES)],
                         axis=0)
    return out.reshape(B, S, D).astype(np.float32)

